# revision 1
# baseline (speedup 1.0000x reference)
"""AdvancedVectorMemory fused kernel for 8 Trainium2 NeuronCores.

Sharding: core c handles batch b = c//4 and heads 4*(c%4) .. 4*(c%4)+3
(data parallel over batch, tensor parallel over heads). Attention runs
flash-style per head pair with fused denominators (ones column in V).
Normalization is deferred past an AllToAll that hands every core the
s-slice (512 rows) it owns for the output projection / gate epilogue.

All matmuls run as float32r (tf32-like) at full PE rate. Softmax exp runs
on the scalar engine directly out of PSUM in [128, 1536] strips.
"""
import sys
import numpy as np

for _p in ('/opt/trn_rl_repo', '/root/.axon_site/_ro/trn_rl_repo'):
    if _p not in sys.path:
        sys.path.insert(0, _p)

B, S, M = 2, 2048, 4096
DM, DK = 1024, 768
H, Dh = 16, 64
NC = 8
GS = 4           # group size (cores per batch)
SC_W = 512       # s-chunk width
N_SC = S // SC_W
N_MT = M // 128  # 32 m-tiles
SSL = S // GS    # per-core s-slice for the epilogue (512)

_PROG = None


def _build_program():
    from concourse import bacc, mybir, tile
    import concourse.bass as bass

    F32 = mybir.dt.float32
    F32R = mybir.dt.float32r
    AF = mybir.ActivationFunctionType
    ALU = mybir.AluOpType

    nc = bacc.Bacc('TRN2', target_bir_lowering=False, debug=False, num_devices=NC)

    def din(name, shape, dt=F32R):
        return nc.dram_tensor(name, shape, dt, kind='ExternalInput').ap()

    qT = din('qT', [DM, S])
    mkT = din('mkT', [DK, M])
    mvT = din('mvT', [DK, M])
    wqT = din('wqT', [DM, 256])
    wkT = din('wkT', [DK, 256])
    wvT = din('wvT', [DK, 256])
    woT = din('woT', [DM, DM])
    wg1T = din('wg1T', [2 * DM, DM])
    wg2T = din('wg2T', [DM, 2])
    qsT = din('qsT', [DM, SSL])
    sel2 = din('sel2', [2, 128])      # row-block indicator for denom bcast
    bc0 = din('bc0', [2, 128])        # row0 = ones (gate broadcast)
    bqv = din('bqv', [2, 128], F32)
    bkv = din('bkv', [2, 128], F32)
    bo2v = din('bo2v', [8, 128], F32)
    bg1v = din('bg1v', [8, 128], F32)
    bg2v = din('bg2v', [2, 1], F32)
    vones = nc.dram_tensor('vones', [128, 8], mybir.dt.bfloat16,
                           kind='ExternalInput').ap()
    zpad = din('zpad', [64, S])
    gidx = nc.dram_tensor('gidx', [4, 128], mybir.dt.int32, kind='ExternalInput').ap()
    didx = nc.dram_tensor('didx', [1, 128], mybir.dt.int32, kind='ExternalInput').ap()
    sel8 = din('sel8', [128, 512])

    out_t = nc.dram_tensor('out_t', [DM, SSL], F32, kind='ExternalOutput').ap()
    import os
    _dbg = os.environ.get('KDBG') == '1'
    if _dbg:
        dbg_rtin = nc.dram_tensor('dbg_rtin', [528, 512], F32, kind='ExternalOutput').ap()
        dbg_rtout = nc.dram_tensor('dbg_rtout', [2112, 512], F32, kind='ExternalOutput').ap()
        dbg_rtn = nc.dram_tensor('dbg_rtn', [128, 8 * SSL], F32, kind='ExternalOutput').ap()
        dbg_dn = nc.dram_tensor('dbg_dn', [2, 4 * 512], F32, kind='ExternalOutput').ap()

    groups = [list(range(GS)), list(range(GS, 2 * GS))]

    with tile.TileContext(nc) as tc:
        with tc.tile_pool(name='consts', bufs=1) as consts, \
             tc.tile_pool(name='dram', bufs=1, space='DRAM') as dram:

            # ---------------- small constants ----------------
            wq_sb = consts.tile([128, 2048], F32R, tag='wq_sb')
            for k in range(8):
                nc.sync.dma_start(out=wq_sb[:, 256 * k:256 * (k + 1)],
                                  in_=wqT[128 * k:128 * (k + 1), :])
            wk_sb = consts.tile([128, 1536], F32R, tag='wk_sb')
            wv_sb = consts.tile([128, 1536], F32R, tag='wv_sb')
            for k in range(6):
                nc.sync.dma_start(out=wk_sb[:, 256 * k:256 * (k + 1)],
                                  in_=wkT[128 * k:128 * (k + 1), :])
                nc.sync.dma_start(out=wv_sb[:, 256 * k:256 * (k + 1)],
                                  in_=wvT[128 * k:128 * (k + 1), :])
            bq_sb = consts.tile([128, 2], F32, tag='bq_sb')
            bk_sb = consts.tile([128, 2], F32, tag='bk_sb')
            for p in range(2):
                nc.sync.dma_start(out=bq_sb[:, p:p + 1], in_=bqv[p:p + 1, :])
                nc.sync.dma_start(out=bk_sb[:, p:p + 1], in_=bkv[p:p + 1, :])
            gidx_sb = []
            for r in range(4):
                gt = consts.tile([128, 1], mybir.dt.int32, tag=f'gidx{r}',
                                 name=f'gidx{r}')
                nc.sync.dma_start(out=gt[:], in_=gidx[r:r + 1, :])
                gidx_sb.append(gt)
            didx8_sb = consts.tile([128, 1], mybir.dt.int32, tag='didx8')
            nc.sync.dma_start(out=didx8_sb[:], in_=didx[0:1, :])
            sel8_sb = consts.tile([128, 512], F32R, tag='sel8_sb')
            nc.sync.dma_start(out=sel8_sb[:], in_=sel8[:])
            sel2_sb = consts.tile([2, 128], F32R, tag='sel2_sb')
            bc0_sb = consts.tile([2, 128], F32R, tag='bc0_sb')
            nc.sync.dma_start(out=sel2_sb[:], in_=sel2[:])
            nc.sync.dma_start(out=bc0_sb[:], in_=bc0[:])

            rt_in = [dram.tile([528, 512], F32, tag=f'rt_in{p}', name=f'rt_in{p}') for p in range(2)]
            rt_out = [dram.tile([4224, 512], F32, tag=f'rt_out{p}', name=f'rt_out{p}') for p in range(2)]

            with tc.tile_pool(name='proj', bufs=1) as proj:
                # ---------------- phase A: projections ----------------
                qt_pair = [proj.tile([128, S], F32R, tag=f'qt_pair{p}',
                                     name=f'qt_pair{p}') for p in range(2)]
                kt_pair = [proj.tile([128, M], F32R, tag=f'kt_pair{p}', name=f'kt_pair{p}') for p in range(2)]
                v_sb = [proj.tile([128, 264], mybir.dt.bfloat16, tag=f'v_sb{mt}', name=f'v_sb{mt}') for mt in range(N_MT)]

                with tc.tile_pool(name='qin', bufs=1) as qin, \
                     tc.tile_pool(name='psA', bufs=1, space='PSUM') as psA:
                    qt_chunks = qin.tile([128, 8 * S], F32R, tag='qt_chunks')
                    for k in range(8):
                        nc.sync.dma_start(out=qt_chunks[:, S * k:S * (k + 1)],
                                          in_=qT[128 * k:128 * (k + 1), :])
                    pqs = [psA.tile([128, 512], F32, tag=f'pq{i}', name=f'pq{i}')
                           for i in range(8)]
                    for k in range(8):
                        for p in range(2):
                            for sc in range(N_SC):
                                nc.tensor.matmul(
                                    pqs[4 * p + sc][:],
                                    wq_sb[:, 256 * k + 128 * p:256 * k + 128 * (p + 1)],
                                    qt_chunks[:, S * k + SC_W * sc:S * k + SC_W * (sc + 1)],
                                    start=(k == 0), stop=(k == 7))
                    for p in range(2):
                        for sc in range(N_SC):
                            nc.vector.tensor_scalar_add(
                                qt_pair[p][:, SC_W * sc:SC_W * (sc + 1)],
                                pqs[4 * p + sc][:], bq_sb[:, p:p + 1])

                with tc.tile_pool(name='mkin', bufs=2) as mkin, \
                     tc.tile_pool(name='psK', bufs=1, space='PSUM') as psK, \
                     tc.tile_pool(name='psV', bufs=1, space='PSUM') as psV, \
                     tc.tile_pool(name='attn', bufs=6) as apool, \
                     tc.tile_pool(name='rtst', bufs=2) as rtst, \
                     tc.tile_pool(name='psQK', bufs=2, space='PSUM') as psQK, \
                     tc.tile_pool(name='psAV', bufs=1, space='PSUM') as psAV:
                    for mc in range(8):  # m blocks of 512
                        mkb = mkin.tile([128, 3072], F32R, tag='mkb')
                        mvb = mkin.tile([128, 3072], F32R, tag='mvb')
                        for k in range(6):
                            nc.sync.dma_start(
                                out=mkb[:, 512 * k:512 * (k + 1)],
                                in_=mkT[128 * k:128 * (k + 1), 512 * mc:512 * (mc + 1)])
                            nc.sync.dma_start(
                                out=mvb[:, 512 * k:512 * (k + 1)],
                                in_=mvT[128 * k:128 * (k + 1), 512 * mc:512 * (mc + 1)])
                        for p in range(2):
                            pk = psK.tile([128, 512], F32, tag='pk')
                            for k in range(6):
                                nc.tensor.matmul(
                                    pk[:],
                                    wk_sb[:, 256 * k + 128 * p:256 * k + 128 * (p + 1)],
                                    mkb[:, 512 * k:512 * (k + 1)],
                                    start=(k == 0), stop=(k == 5))
                            nc.vector.tensor_scalar_add(
                                kt_pair[p][:, 512 * mc:512 * (mc + 1)], pk[:],
                                bk_sb[:, p:p + 1])
                        for ml in range(4):
                            mt = 4 * mc + ml
                            pv = psV.tile([128, 256], F32, tag='pv')
                            for k in range(6):
                                nc.tensor.matmul(
                                    pv[:],
                                    mvb[:, 512 * k + 128 * ml:512 * k + 128 * (ml + 1)],
                                    wv_sb[:, 256 * k:256 * (k + 1)],
                                    start=(k == 0), stop=(k == 5))
                            vh = v_sb[mt].rearrange('p (h c) -> p h c', h=4)
                            nc.sync.dma_start(
                                out=vh[:, :, 64:66],
                                in_=vones[:].rearrange('p (h c) -> p h c', h=4))
                            nc.vector.tensor_copy(
                                vh[:, :, 0:64],
                                pv[:].rearrange('p (h d) -> p h d', h=4))

                    # ---------------- attention ----------------
                    for p in range(2):
                        for sc in range(N_SC):
                            accA = psAV.tile([66, 512], F32, tag='accA')
                            accB = psAV.tile([66, 512], F32, tag='accB')
                            for mt in range(N_MT):
                                tAB = psQK.tile([128, 1024], F32, tag='tAB')
                                nc.tensor.matmul(
                                    tAB[:, 0:512],
                                    kt_pair[p][0:64, 128 * mt:128 * (mt + 1)],
                                    qt_pair[p][0:64, SC_W * sc:SC_W * (sc + 1)],
                                    start=True, stop=True)
                                nc.tensor.matmul(
                                    tAB[:, 512:1024],
                                    kt_pair[p][64:128, 128 * mt:128 * (mt + 1)],
                                    qt_pair[p][64:128, SC_W * sc:SC_W * (sc + 1)],
                                    start=True, stop=True)
                                atAB = apool.tile([128, 1024], mybir.dt.bfloat16,
                                                  tag='at')
                                nc.scalar.activation(atAB[:], tAB[:], AF.Exp)
                                nc.tensor.matmul(
                                    accA[:],
                                    v_sb[mt][:, 66 * (2 * p):66 * (2 * p) + 66],
                                    atAB[:, 0:512],
                                    start=(mt == 0), stop=(mt == N_MT - 1))
                                nc.tensor.matmul(
                                    accB[:],
                                    v_sb[mt][:, 66 * (2 * p + 1):66 * (2 * p + 1) + 66],
                                    atAB[:, 512:1024],
                                    start=(mt == 0), stop=(mt == N_MT - 1))
                            rt_tA = rtst.tile([66, 512], F32, tag='rt_tA')
                            nc.vector.tensor_copy(rt_tA[:], accA[0:66, :])
                            rt_tB = rtst.tile([66, 512], F32, tag='rt_tB')
                            nc.vector.tensor_copy(rt_tB[:], accB[0:66, :])
                            nc.sync.dma_start(
                                out=rt_in[p][132 * sc:132 * sc + 66, :], in_=rt_tA[:])
                            nc.sync.dma_start(
                                out=rt_in[p][132 * sc + 66:132 * sc + 132, :],
                                in_=rt_tB[:])
                            nc.gpsimd.collective_compute(
                                'AllGather', ALU.bypass,
                                replica_groups=[list(range(NC))],
                                ins=[rt_in[p][132 * sc:132 * (sc + 1), :].opt()],
                                outs=[rt_out[p][1056 * sc:1056 * (sc + 1), :].opt()])

            # ---------------- epilogue (own s-slice) ----------------
            with tc.tile_pool(name='ep', bufs=1) as ep, \
                 tc.tile_pool(name='ept', bufs=3) as ept:
                qs_sb = ep.tile([128, 8 * SSL], F32R, tag='qs_sb')
                for k in range(8):
                    nc.scalar.dma_start(out=qs_sb[:, SSL * k:SSL * (k + 1)],
                                        in_=qsT[128 * k:128 * (k + 1), :])
                bo2_sb = ep.tile([128, 8], F32, tag='bo2_sb')
                bg1_sb = ep.tile([128, 8], F32, tag='bg1_sb')
                for k in range(8):
                    nc.scalar.dma_start(out=bo2_sb[:, k:k + 1], in_=bo2v[k:k + 1, :])
                    nc.scalar.dma_start(out=bg1_sb[:, k:k + 1], in_=bg1v[k:k + 1, :])
                bg2_sb = ep.tile([2, 1], F32, tag='bg2_sb')
                nc.scalar.dma_start(out=bg2_sb[:], in_=bg2v[:])
                wg2_sb = ep.tile([128, 16], F32R, tag='wg2_sb')
                for k in range(8):
                    nc.scalar.dma_start(out=wg2_sb[:, 2 * k:2 * (k + 1)],
                                        in_=wg2T[128 * k:128 * (k + 1), :])

                # fetch + normalize retrieved k-chunks
                ctx_wo = tc.tile_pool(name='wo1', bufs=1)
                wo1 = ctx_wo.__enter__()
                ctx_w1s = tc.tile_pool(name='w1s', bufs=4)
                w1s = ctx_w1s.__enter__()
                ctx_psG = tc.tile_pool(name='psG', bufs=1, space='PSUM')
                psG = ctx_psG.__enter__()
                ctx_psWo = tc.tile_pool(name='psWo', bufs=3, space='PSUM')
                psWo = ctx_psWo.__enter__()
                wo_sb = wo1.tile([128, 8 * DM], F32R, tag='wo_sb')
                for k in range(8):
                    nc.scalar.dma_start(out=wo_sb[:, DM * k:DM * (k + 1)],
                                        in_=woT[128 * k:128 * (k + 1), :])
                rd2 = {}
                for pp in range(2):
                    dgt = ept.tile([128, 512], F32, tag='dgt', name=f'dgt{pp}')
                    nc.gpsimd.indirect_dma_start(
                        out=dgt[:], out_offset=None, in_=rt_out[pp][:],
                        in_offset=bass.IndirectOffsetOnAxis(ap=didx8_sb[:], axis=0))
                    rdf = ept.tile([128, 512], F32, tag='rdf', name=f'rdf{pp}')
                    nc.vector.reciprocal(rdf[:], dgt[:])
                    rd = ep.tile([128, 512], F32R, tag=f'rd{pp}', name=f'rd{pp}')
                    nc.vector.tensor_copy(rd[:], rdf[:])
                    rd2[pp] = rd
                rtn = wo1.tile([128, 8 * SSL], F32R, tag='rtn')
                for kc in [0, 2, 4, 6, 1, 3, 5, 7]:
                    r, pp = kc // 2, kc % 2
                    raw = ept.tile([128, 512], F32, tag='raw')
                    nc.gpsimd.indirect_dma_start(
                        out=raw[:], out_offset=None, in_=rt_out[pp][:],
                        in_offset=bass.IndirectOffsetOnAxis(ap=gidx_sb[r][:], axis=0))
                    bcp = psWo.tile([128, 512], F32, tag='bcp', bufs=1)
                    nc.tensor.matmul(bcp[:], sel8_sb[:, 128 * r:128 * (r + 1)],
                                     rd2[pp][:], start=True, stop=True)
                    nc.vector.tensor_tensor(
                        rtn[:, SSL * kc:SSL * (kc + 1)], raw[:], bcp[:], ALU.mult)

                if _dbg:
                    nc.sync.dma_start(out=dbg_rtin[:], in_=rt_in[0][:])
                    nc.sync.dma_start(out=dbg_rtout[:], in_=rt_out[0][:])
                    nc.sync.dma_start(out=dbg_rtn[:], in_=rtn[:].bitcast(F32))

                # Wo projection + bias
                oT = ep.tile([128, 8 * SSL], F32R, tag='oT')
                for dt in range(8):
                    po = psWo.tile([128, 512], F32, tag='po')
                    for i, kc in enumerate([0, 2, 4, 6, 1, 3, 5, 7]):
                        nc.tensor.matmul(
                            po[:], wo_sb[:, DM * kc + 128 * dt:DM * kc + 128 * (dt + 1)],
                            rtn[:, SSL * kc:SSL * (kc + 1)],
                            start=(i == 0), stop=(i == 7))
                    nc.vector.tensor_scalar_add(
                        oT[:, SSL * dt:SSL * (dt + 1)], po[:], bo2_sb[:, dt:dt + 1])
                ctx_psWo.__exit__(None, None, None)

                # gate MLP: g = silu(Wg1 @ [q; o] + bg1); Wg1 streamed kc-outer
                sl = ep.tile([128, 8 * SSL], F32R, tag='sl')

                def wg1_kcs(half, pgs, kcs):
                    dts = [4 * half + i for i in range(4)]
                    for kc in kcs:
                        w1t = w1s.tile([128, 512], F32R, tag='w1t', name='w1t')
                        nc.scalar.dma_start(
                            out=w1t[:],
                            in_=wg1T[128 * kc:128 * (kc + 1),
                                     512 * half:512 * (half + 1)])
                        rhs = (qs_sb[:, SSL * kc:SSL * (kc + 1)] if kc < 8
                               else oT[:, SSL * (kc - 8):SSL * (kc - 7)])
                        for i, dt in enumerate(dts):
                            nc.tensor.matmul(
                                pgs[i][:], w1t[:, 128 * i:128 * (i + 1)],
                                rhs, start=(kc == 0), stop=(kc == 15))

                def wg1_silu(half, pgs):
                    for i in range(4):
                        dt = 4 * half + i
                        sg = ept.tile([128, 512], F32, tag='sg', name='sg')
                        nc.scalar.activation(sg[:], pgs[i][:], AF.Sigmoid,
                                             bias=bg1_sb[:, dt:dt + 1])
                        gg = ept.tile([128, 512], F32, tag='gg', name='gg')
                        nc.vector.tensor_scalar_add(gg[:], pgs[i][:],
                                                    bg1_sb[:, dt:dt + 1])
                        nc.vector.tensor_tensor(
                            sl[:, SSL * dt:SSL * (dt + 1)], gg[:], sg[:], ALU.mult)

                pgs0 = [psG.tile([128, 512], F32, tag=f'pg{i}', name=f'pg0_{i}')
                        for i in range(4)]
                wg1_kcs(0, pgs0, range(0, 8))  # query part: runs in the AG hole

                wg1_kcs(0, pgs0, range(8, 16))
                wg1_silu(0, pgs0)
                pgs1 = [psG.tile([128, 512], F32, tag=f'pg{i}', name=f'pg1_{i}')
                        for i in range(4)]
                wg1_kcs(1, pgs1, range(0, 16))
                wg1_silu(1, pgs1)
                ctx_psG.__exit__(None, None, None)
                ctx_w1s.__exit__(None, None, None)
                ctx_wo.__exit__(None, None, None)

                # gate scalar: sigmoid(Wg2 @ sl + bg2), broadcast to 128 rows
                ctx_psT = tc.tile_pool(name='psT', bufs=1, space='PSUM')
                psT = ctx_psT.__enter__()
                pgt = psT.tile([2, 512], F32, tag='pgt')
                for kc in range(8):
                    nc.tensor.matmul(pgt[:], wg2_sb[:, 2 * kc:2 * (kc + 1)],
                                     sl[:, SSL * kc:SSL * (kc + 1)],
                                     start=(kc == 0), stop=(kc == 7))
                gate = ep.tile([2, 512], F32R, tag='gate')
                nc.scalar.activation(gate[:], pgt[:], AF.Sigmoid, bias=bg2_sb[:])
                gb = psT.tile([128, 512], F32, tag='gb')
                nc.tensor.matmul(gb[:], bc0_sb[:], gate[:], start=True, stop=True)
                gbs = ep.tile([128, 512], F32, tag='gbs')
                nc.vector.tensor_copy(gbs[:], gb[:])

                # out = q + gate * o
                for dt in range(8):
                    go = ept.tile([128, 512], F32, tag='go')
                    nc.vector.tensor_tensor(
                        go[:], gbs[:], oT[:, SSL * dt:SSL * (dt + 1)].bitcast(F32),
                        ALU.mult)
                    fo = ept.tile([128, 512], F32, tag='fo')
                    nc.vector.tensor_tensor(
                        fo[:], go[:], qs_sb[:, SSL * dt:SSL * (dt + 1)].bitcast(F32),
                        ALU.add)
                    nc.sync.dma_start(out=out_t[128 * dt:128 * (dt + 1), :], in_=fo[:])
                ctx_psT.__exit__(None, None, None)

    nc.compile()
    return nc


def _shard(inputs):
    import ml_dtypes
    _bf16 = ml_dtypes.bfloat16
    q = np.asarray(inputs['query'], np.float32)
    mk = np.asarray(inputs['memory_keys'], np.float32)
    mv = np.asarray(inputs['memory_values'], np.float32)
    Wq = np.asarray(inputs['Wq'], np.float32); bq = np.asarray(inputs['bq'], np.float32)
    Wk = np.asarray(inputs['Wk'], np.float32); bk = np.asarray(inputs['bk'], np.float32)
    Wv = np.asarray(inputs['Wv'], np.float32); bv = np.asarray(inputs['bv'], np.float32)
    Wo = np.asarray(inputs['Wo'], np.float32); bo = np.asarray(inputs['bo'], np.float32)
    Wg1 = np.asarray(inputs['Wg1'], np.float32); bg1 = np.asarray(inputs['bg1'], np.float32)
    Wg2 = np.asarray(inputs['Wg2'], np.float32); bg2 = np.asarray(inputs['bg2'], np.float32)

    scale = Dh ** -0.5
    bo2 = bo + Wo @ bv
    sel2 = np.zeros((2, 128), np.float32)
    sel2[0, 0:64] = 1.0
    sel2[1, 64:128] = 1.0
    bc0 = np.zeros((2, 128), np.float32)
    bc0[0, :] = 1.0
    wg2T = np.zeros((DM, 2), np.float32)
    wg2T[:, 0] = Wg2[0]
    bg2v = np.zeros((2, 1), np.float32)
    bg2v[:, 0] = bg2[0]
    _sel8 = np.zeros((128, 512), np.float32)
    for _r in range(4):
        for _j in range(128):
            _sel8[2 * _r + _j // 64, 128 * _r + _j] = 1.0

    qT_b = [np.ascontiguousarray(q[b].T) for b in range(B)]
    mkT_b = [np.ascontiguousarray(mk[b].T) for b in range(B)]
    mvT_b = [np.ascontiguousarray(mv[b].T) for b in range(B)]

    in_maps = []
    for c in range(NC):
        b, g = c // GS, c % GS
        hs = slice(64 * 4 * g, 64 * (4 * g + 4))  # rows of W for this core's 4 heads
        in_maps.append({
            'qT': qT_b[b],
            'mkT': mkT_b[b],
            'mvT': mvT_b[b],
            'wqT': np.ascontiguousarray((Wq[hs] * scale).T),
            'wkT': np.ascontiguousarray(Wk[hs].T),
            'wvT': np.ascontiguousarray(Wv[hs].T),
            'woT': np.ascontiguousarray(Wo.T),
            'wg1T': np.ascontiguousarray(Wg1.T),
            'wg2T': wg2T,
            'qsT': np.ascontiguousarray(q[b].T[:, SSL * g:SSL * (g + 1)]),
            'sel2': sel2,
            'vones': np.ascontiguousarray(np.tile([1.0, 0.0], 4)[None, :].repeat(128, 0).astype(_bf16)),
            'zpad': np.zeros((64, S), np.float32),
            'bc0': bc0,
            'bqv': np.ascontiguousarray((bq[hs] * scale).reshape(2, 128)),
            'bkv': np.ascontiguousarray(bk[hs].reshape(2, 128)),
            'bo2v': np.ascontiguousarray(bo2.reshape(8, 128)),
            'bg1v': np.ascontiguousarray(bg1.reshape(8, 128)),
            'bg2v': bg2v,
            'gidx': np.asarray(
                [[1056 * g + 132 * (4 * b + r) + 66 * (j // 64) + (j % 64)
                  for j in range(128)] for r in range(4)], np.int32),
            'didx': np.asarray(
                [[1056 * g + 132 * (4 * b + (j // 2)) + 66 * (j % 2) + 64 if j < 8
                  else 0 for j in range(128)]], np.int32),
            'sel8': _sel8,
        })
    return in_maps


def _run(inputs, trace=False):
    global _PROG
    from concourse.bass_utils import run_bass_kernel_spmd
    if _PROG is None:
        _PROG = _build_program()
    in_maps = _shard(inputs)
    res = run_bass_kernel_spmd(_PROG, in_maps, list(range(NC)), trace=trace)
    out = np.empty((B, S, DM), np.float32)
    for c in range(NC):
        b, g = c // GS, c % GS
        out[b, SSL * g:SSL * (g + 1), :] = res.results[c]['out_t'].T
    return out, res


def kernel(**inputs) -> np.ndarray:
    out, _ = _run(inputs, trace=False)
    return out

